# revision 18
# baseline (speedup 1.0000x reference)
"""Trainium2 Bass kernel for nn_Net_25254407701209 (dense_mlp).

Reference math (B=4096, N_OBS=64, H=256):
    z   = x w1^T + b1;  y1 = z^2
    z12 = y1 w2a^T + b2a; z22 = y1 w2b^T + b2b; y2 = z12*z22
    z13 = y2 w3a^T + b3a; z23 = x w3s^T + b3s;  y3 = z13*z23
    b   = y3 wout^T;  yy = scalar broadcast
The reference's full-Jacobian chain collapses to a forward-mode JVP with
tangent xdot:
    v1 = 2 z * (xdot w1^T)
    t1 = z12 * (v1 w2b^T);  t2 = z22 * (v1 w2a^T)      (v2 = t1 + t2)
    g  = w3a v2 = w3a t1 + w3a t2     <- add folded into PSUM accumulation
    t3 = z13 * (xdot w3s^T);  t4 = z23 * g             (v3 = t3 + t4)
    bdot = wout . v3 = wout.t3 + wout.t4               <- folded likewise

Sharding: pure data-parallel, batch 4096 -> 8 cores x 512 rows. Feature-on-
partition layout: each [512, 256] activation is one [128, 1024] tile (free
block m = features 128m..128m+127). Matmuls in float32r (1 cycle/row).
All weights arrive in ONE packed DMA (BIGW), x/xdot in another (XX, with
x on partitions 0-63 and xdot on 64-127 so the K=64 layer-1 matmuls
row-pack into concurrent array halves). Biases fold into PSUM->SBUF
evictions. yy is an input-independent broadcast, assembled host-side.
"""
import sys

if "/opt/trn_rl_repo" not in sys.path:
    sys.path.insert(0, "/opt/trn_rl_repo")

import numpy as np

N_CORES = 8
B, NOBS, H = 4096, 64, 256
BL = B // N_CORES
P = 128
N = BL

# BIGW column layout
C_W2AT = 0          # [128, 512] w2a^T (k0 | k1)
C_W2BT = 512
C_W3AT = 1024
C_W1X = 1536        # [64,256] w1^T duplicated on partitions 0-63 and 64-127
C_W3X = 1792        # w3s^T likewise
C_WOUT = 2048       # [128, 2] wout^T k-halves
C_BC = 2050         # [128, 10] biases: col 2j+m = bias_j[128m:128(m+1)]
C_TOT = 2060

TRACE = False
TRACE_KW = {}
LAST = None
_CACHE = {}


def _build():
    import concourse.bacc as bacc
    import concourse.mybir as mybir
    import concourse.tile as tile

    F32 = mybir.dt.float32
    F32R = mybir.dt.float32r
    AF = mybir.ActivationFunctionType
    MUL = mybir.AluOpType.mult
    ADD = mybir.AluOpType.add

    nc = bacc.Bacc("TRN2", target_bir_lowering=False, debug=False, num_devices=N_CORES)

    d_xx = nc.dram_tensor("xx", [P, N], F32R, kind="ExternalInput").ap()
    d_bigw = nc.dram_tensor("bigw", [P, C_TOT], F32R, kind="ExternalInput").ap()

    d_y = nc.dram_tensor("ydram", [P, 2 * N], F32R, kind="ExternalOutput").ap()
    d_b = nc.dram_tensor("bdram", [1, N], F32, kind="ExternalOutput").ap()
    d_bd = nc.dram_tensor("bddram", [1, N], F32, kind="ExternalOutput").ap()

    MSL = [slice(0, P), slice(P, 2 * P)]
    FSL = [slice(0, N), slice(N, 2 * N)]

    with tile.TileContext(nc) as tc:
        with (
            tc.tile_pool(name="io", bufs=1) as io,
            tc.tile_pool(name="act", bufs=1) as act,
            tc.tile_pool(name="psum", bufs=4, space="PSUM") as pp,
        ):
            t_xx = io.tile([P, N], F32R, name="t_xx")
            t_bigw = io.tile([P, C_TOT], F32R, name="t_bigw")
            # spread input DMA issues across the two HWDGE queues (SP + ACT)
            nc.sync.dma_start(out=t_xx[:], in_=d_xx[:])
            # early block: layer-1/3 skip weights + wout + biases (~134KB)
            nc.scalar.dma_start(out=t_bigw[:, C_W1X:C_TOT], in_=d_bigw[:, C_W1X:C_TOT])
            # late weights, one DMA each in order of first use, queues
            # alternated so transfers overlap
            nc.gpsimd.dma_start(out=t_bigw[:, C_W2AT:C_W2BT], in_=d_bigw[:, C_W2AT:C_W2BT])
            nc.gpsimd.dma_start(out=t_bigw[:, C_W2BT:C_W3AT], in_=d_bigw[:, C_W2BT:C_W3AT])
            nc.gpsimd.dma_start(out=t_bigw[:, C_W3AT:C_W1X], in_=d_bigw[:, C_W3AT:C_W1X])

            def wsl(base, lo, hi, cols):
                return t_bigw[lo:hi, base + cols.start:base + cols.stop]

            def bcol(j, m):
                c = C_BC + 2 * j + m
                return t_bigw[:, c:c + 1].bitcast(F32)

            # PE warmup: two fp32 matmuls (f32r work does not reliably lift
            # the HAM clock gate) on the early weight block, the first data
            # to arrive. ~2.5us of PE busy -> warm clock for the real work.
            p_warm = pp.tile([P, N], F32, name="p_warm", tag="ps")
            nc.tensor.matmul(p_warm[:], t_bigw[:, C_W1X:C_W1X + P].bitcast(F32),
                             t_bigw[:, C_W1X:C_W1X + N].bitcast(F32),
                             start=True, stop=True)
            nc.tensor.matmul(p_warm[:, 0:H], t_bigw[:, C_W1X:C_W1X + P].bitcast(F32),
                             t_bigw[:, C_W1X:C_W1X + H].bitcast(F32),
                             start=True, stop=True)

            # ACT table preload: tiny activation off the const AP so the
            # 1.3us ACT_TABLE_LOAD happens during the input DMA, not on the
            # critical path of the first real eviction.
            t_warm = act.tile([1, 1], F32, name="t_warm")
            nc.scalar.activation(t_warm[:], nc.const_aps.aps[(F32, 0.0)][0:1, 0:1],
                                 AF.Square, bias=0.0, scale=1.0)

            # ---- phase 1: K=64 row-packed matmuls ----
            p_z = pp.tile([P, 2 * N], F32, name="p_z", tag="ps")
            p_u1 = pp.tile([P, 2 * N], F32, name="p_u1", tag="ps")
            p_z23 = pp.tile([P, 2 * N], F32, name="p_z23", tag="ps")
            for m in range(2):
                nc.tensor.matmul(p_z[:, FSL[m]], wsl(C_W1X, 0, NOBS, MSL[m]),
                                 t_xx[0:NOBS, :], start=True, stop=True)
                nc.tensor.matmul(p_u1[:, FSL[m]], wsl(C_W1X, NOBS, P, MSL[m]),
                                 t_xx[NOBS:P, :], start=True, stop=True)
            for m in range(2):
                nc.tensor.matmul(p_z23[:, FSL[m]], wsl(C_W3X, 0, NOBS, MSL[m]),
                                 t_xx[0:NOBS, :], start=True, stop=True)

            t_y1 = act.tile([P, 2 * N], F32R, name="t_y1")
            t_ze = act.tile([P, 2 * N], F32, name="t_ze")
            t_z23e = act.tile([P, 2 * N], F32, name="t_z23e")
            for m in range(2):
                nc.scalar.activation(t_y1[:, FSL[m]], p_z[:, FSL[m]], AF.Square,
                                     bias=bcol(0, m), scale=1.0)
            t_v1 = act.tile([P, 2 * N], F32R, name="t_v1")
            for m in range(2):
                nc.vector.tensor_scalar(t_ze[:, FSL[m]], p_z[:, FSL[m]],
                                        bcol(0, m), None, op0=ADD)
                nc.vector.scalar_tensor_tensor(t_v1[:, FSL[m]], t_ze[:, FSL[m]],
                                               2.0, p_u1[:, FSL[m]],
                                               op0=MUL, op1=MUL)

            # ---- phase 2: layer 2 ----
            p_z12 = pp.tile([P, 2 * N], F32, name="p_z12", tag="ps")
            p_z22 = pp.tile([P, 2 * N], F32, name="p_z22", tag="ps")
            p_a = pp.tile([P, 2 * N], F32, name="p_a", tag="ps")
            p_b = pp.tile([P, 2 * N], F32, name="p_b", tag="ps")
            for m in range(2):
                nc.tensor.matmul(p_z12[:, FSL[m]], wsl(C_W2AT, 0, P, MSL[m]),
                                 t_y1[:, FSL[0]], start=True, stop=False)
                nc.tensor.matmul(p_z12[:, FSL[m]],
                                 wsl(C_W2AT, 0, P, slice(H + MSL[m].start, H + MSL[m].stop)),
                                 t_y1[:, FSL[1]], start=False, stop=True)
                nc.tensor.matmul(p_z22[:, FSL[m]], wsl(C_W2BT, 0, P, MSL[m]),
                                 t_y1[:, FSL[0]], start=True, stop=False)
                nc.tensor.matmul(p_z22[:, FSL[m]],
                                 wsl(C_W2BT, 0, P, slice(H + MSL[m].start, H + MSL[m].stop)),
                                 t_y1[:, FSL[1]], start=False, stop=True)
            for m in range(2):
                nc.tensor.matmul(p_a[:, FSL[m]], wsl(C_W2BT, 0, P, MSL[m]),
                                 t_v1[:, FSL[0]], start=True, stop=False)
                nc.tensor.matmul(p_a[:, FSL[m]],
                                 wsl(C_W2BT, 0, P, slice(H + MSL[m].start, H + MSL[m].stop)),
                                 t_v1[:, FSL[1]], start=False, stop=True)
                nc.tensor.matmul(p_b[:, FSL[m]], wsl(C_W2AT, 0, P, MSL[m]),
                                 t_v1[:, FSL[0]], start=True, stop=False)
                nc.tensor.matmul(p_b[:, FSL[m]],
                                 wsl(C_W2AT, 0, P, slice(H + MSL[m].start, H + MSL[m].stop)),
                                 t_v1[:, FSL[1]], start=False, stop=True)

            t_z12e = act.tile([P, 2 * N], F32, name="t_z12e")
            t_z22e = act.tile([P, 2 * N], F32, name="t_z22e")
            for m in range(2):
                nc.scalar.activation(t_z12e[:, FSL[m]], p_z12[:, FSL[m]], AF.Identity,
                                     bias=bcol(1, m), scale=1.0)
                nc.vector.tensor_scalar(t_z22e[:, FSL[m]], p_z22[:, FSL[m]],
                                        bcol(2, m), None, op0=ADD)
            for m in range(2):
                nc.scalar.activation(t_z23e[:, FSL[m]], p_z23[:, FSL[m]], AF.Identity,
                                     bias=bcol(4, m), scale=1.0)
            t_y2 = act.tile([P, 2 * N], F32R, name="t_y2")
            t_t1 = act.tile([P, 2 * N], F32R, name="t_t1")
            t_t2 = act.tile([P, 2 * N], F32R, name="t_t2")
            for m in range(2):
                nc.vector.tensor_mul(t_t1[:, FSL[m]], t_z12e[:, FSL[m]], p_a[:, FSL[m]])
            for m in range(2):
                nc.vector.tensor_mul(t_t2[:, FSL[m]], t_z22e[:, FSL[m]], p_b[:, FSL[m]])
            for m in range(2):
                nc.gpsimd.tensor_mul(t_y2[:, FSL[m]], t_z12e[:, FSL[m]], t_z22e[:, FSL[m]])

            # ---- phase 3: layer 3 (u3s joins here to ease PSUM pressure) ----
            p_z13 = pp.tile([P, 2 * N], F32, name="p_z13", tag="ps")
            p_g = pp.tile([P, 2 * N], F32, name="p_g", tag="ps")
            p_u3s = pp.tile([P, 2 * N], F32, name="p_u3s", tag="ps")
            for m in range(2):
                nc.tensor.matmul(p_u3s[:, FSL[m]], wsl(C_W3X, NOBS, P, MSL[m]),
                                 t_xx[NOBS:P, :], start=True, stop=True)
            for m in range(2):
                nc.tensor.matmul(p_z13[:, FSL[m]], wsl(C_W3AT, 0, P, MSL[m]),
                                 t_y2[:, FSL[0]], start=True, stop=False)
                nc.tensor.matmul(p_z13[:, FSL[m]],
                                 wsl(C_W3AT, 0, P, slice(H + MSL[m].start, H + MSL[m].stop)),
                                 t_y2[:, FSL[1]], start=False, stop=True)
                nc.tensor.matmul(p_g[:, FSL[m]], wsl(C_W3AT, 0, P, MSL[m]),
                                 t_t1[:, FSL[0]], start=True, stop=False)
                nc.tensor.matmul(p_g[:, FSL[m]],
                                 wsl(C_W3AT, 0, P, slice(H + MSL[m].start, H + MSL[m].stop)),
                                 t_t1[:, FSL[1]], start=False, stop=False)
                nc.tensor.matmul(p_g[:, FSL[m]], wsl(C_W3AT, 0, P, MSL[m]),
                                 t_t2[:, FSL[0]], start=False, stop=False)
                nc.tensor.matmul(p_g[:, FSL[m]],
                                 wsl(C_W3AT, 0, P, slice(H + MSL[m].start, H + MSL[m].stop)),
                                 t_t2[:, FSL[1]], start=False, stop=True)

            t_z13e = act.tile([P, 2 * N], F32, name="t_z13e")
            for m in range(2):
                nc.scalar.activation(t_z13e[:, FSL[m]], p_z13[:, FSL[m]], AF.Identity,
                                     bias=bcol(3, m), scale=1.0)
            t_y3 = act.tile([P, 2 * N], F32R, name="t_y3")
            t_t3 = act.tile([P, 2 * N], F32R, name="t_t3")
            t_t4 = act.tile([P, 2 * N], F32R, name="t_t4")
            for m in range(2):
                nc.vector.tensor_mul(t_t3[:, FSL[m]], t_z13e[:, FSL[m]], p_u3s[:, FSL[m]])
            for m in range(2):
                nc.gpsimd.tensor_mul(t_y3[:, FSL[m]], t_z13e[:, FSL[m]], t_z23e[:, FSL[m]])
            for m in range(2):
                nc.vector.tensor_mul(t_t4[:, FSL[m]], t_z23e[:, FSL[m]], p_g[:, FSL[m]])
            nc.sync.dma_start(out=d_y[:], in_=t_y3[:])

            # ---- phase 4: wout contractions (M=1) ----
            p_bout = pp.tile([1, N], F32, name="p_bout", tag="ps")
            p_bd = pp.tile([1, N], F32, name="p_bd", tag="ps")
            wo = [t_bigw[:, C_WOUT + k:C_WOUT + k + 1] for k in range(2)]
            nc.tensor.matmul(p_bd[0:1, :], wo[0], t_t3[:, FSL[0]], start=True, stop=False)
            nc.tensor.matmul(p_bd[0:1, :], wo[1], t_t3[:, FSL[1]], start=False, stop=False)
            nc.tensor.matmul(p_bout[0:1, :], wo[0], t_y3[:, FSL[0]], start=True, stop=False)
            nc.tensor.matmul(p_bout[0:1, :], wo[1], t_y3[:, FSL[1]], start=False, stop=True)
            nc.tensor.matmul(p_bd[0:1, :], wo[0], t_t4[:, FSL[0]], start=False, stop=False)
            nc.tensor.matmul(p_bd[0:1, :], wo[1], t_t4[:, FSL[1]], start=False, stop=True)

            t_brow = act.tile([1, N], F32, name="t_brow")
            t_bdrow = act.tile([1, N], F32, name="t_bdrow")
            nc.scalar.copy(t_brow[:], p_bout[0:1, :])
            nc.vector.tensor_copy(t_bdrow[:], p_bd[0:1, :])
            nc.sync.dma_start(out=d_b[:], in_=t_brow[:])
            nc.sync.dma_start(out=d_bd[:], in_=t_bdrow[:])

    nc.compile()
    return nc


def kernel(x, xdot, w1, b1, w2a, b2a, w2b, b2b, w3a, b3a, w3s, b3s, wout, scalar):
    from concourse.bass_utils import run_bass_kernel_spmd

    global LAST
    if "nc" not in _CACHE:
        _CACHE["nc"] = _build()
    nc = _CACHE["nc"]

    f = np.float32
    x = np.asarray(x, f)
    xdot = np.asarray(xdot, f)
    sval = np.asarray(scalar, f).reshape(-1)[0]

    xt_full = np.ascontiguousarray(x.T)
    xdt_full = np.ascontiguousarray(xdot.T)

    def ksplit(w):  # [H, H] -> [128, 512] (k0 | k1)
        wt = np.asarray(w, f).T
        return np.concatenate([wt[:P], wt[P:]], axis=1)

    bigw = np.zeros((P, C_TOT), f)
    bigw[:, C_W2AT:C_W2AT + 2 * H] = ksplit(w2a)
    bigw[:, C_W2BT:C_W2BT + 2 * H] = ksplit(w2b)
    bigw[:, C_W3AT:C_W3AT + 2 * H] = ksplit(w3a)
    w1t = np.asarray(w1, f).T
    w3st = np.asarray(w3s, f).T
    bigw[:NOBS, C_W1X:C_W1X + H] = w1t
    bigw[NOBS:, C_W1X:C_W1X + H] = w1t
    bigw[:NOBS, C_W3X:C_W3X + H] = w3st
    bigw[NOBS:, C_W3X:C_W3X + H] = w3st
    wo = np.asarray(wout, f)[0]
    bigw[:, C_WOUT] = wo[:P]
    bigw[:, C_WOUT + 1] = wo[P:]
    for j, bias in enumerate((b1, b2a, b2b, b3a, b3s)):
        bb = np.asarray(bias, f)
        bigw[:, C_BC + 2 * j] = bb[:P]
        bigw[:, C_BC + 2 * j + 1] = bb[P:]
    bigw = np.ascontiguousarray(bigw)

    in_maps = []
    for c in range(N_CORES):
        sl = slice(c * BL, (c + 1) * BL)
        xx = np.empty((P, BL), f)
        xx[:NOBS] = xt_full[:, sl]
        xx[NOBS:] = xdt_full[:, sl]
        in_maps.append({"xx": xx, "bigw": bigw})

    res = run_bass_kernel_spmd(
        nc, in_maps, core_ids=list(range(N_CORES)),
        trace=TRACE, **TRACE_KW)
    LAST = res

    yb = np.empty((B, 1), f)
    ybdot = np.empty((B,), f)
    yfull = np.empty((B, H), f)
    for c in range(N_CORES):
        sl = slice(c * BL, (c + 1) * BL)
        r = res.results[c]
        yb[sl, 0] = r["bdram"][0]
        ybdot[sl] = r["bddram"][0]
        yfull[sl, :P] = r["ydram"][:, :BL].T
        yfull[sl, P:] = r["ydram"][:, BL:].T
    yyfull = np.broadcast_to(np.float32(sval), (B, NOBS)).copy()
    yyfull += x * 0
    return yb, ybdot, yfull, yyfull


# revision 19
# speedup vs baseline: 1.0687x; 1.0687x over previous
"""Trainium2 Bass kernel for nn_Net_25254407701209 (dense_mlp).

Reference math (B=4096, N_OBS=64, H=256):
    z   = x w1^T + b1;  y1 = z^2
    z12 = y1 w2a^T + b2a; z22 = y1 w2b^T + b2b; y2 = z12*z22
    z13 = y2 w3a^T + b3a; z23 = x w3s^T + b3s;  y3 = z13*z23
    b   = y3 wout^T;  yy = scalar broadcast
The reference's full-Jacobian chain collapses to a forward-mode JVP with
tangent xdot:
    v1 = 2 z * (xdot w1^T)
    t1 = z12 * (v1 w2b^T);  t2 = z22 * (v1 w2a^T)      (v2 = t1 + t2)
    g  = w3a v2 = w3a t1 + w3a t2     <- add folded into PSUM accumulation
    t3 = z13 * (xdot w3s^T);  t4 = z23 * g             (v3 = t3 + t4)
    bdot = wout . v3 = wout.t3 + wout.t4               <- folded likewise

Sharding: pure data-parallel, batch 4096 -> 8 cores x 512 rows. Feature-on-
partition layout: each [512, 256] activation is one [128, 1024] tile (free
block m = features 128m..128m+127). Matmuls in float32r (1 cycle/row).
All weights arrive in ONE packed DMA (BIGW), x/xdot in another (XX, with
x on partitions 0-63 and xdot on 64-127 so the K=64 layer-1 matmuls
row-pack into concurrent array halves). Biases fold into PSUM->SBUF
evictions. yy is an input-independent broadcast, assembled host-side.
"""
import sys

if "/opt/trn_rl_repo" not in sys.path:
    sys.path.insert(0, "/opt/trn_rl_repo")

import numpy as np

N_CORES = 8
B, NOBS, H = 4096, 64, 256
BL = B // N_CORES
P = 128
N = BL

# BIGW column layout
C_W2AT = 0          # [128, 512] w2a^T (k0 | k1)
C_W2BT = 512
C_W3AT = 1024
C_W1X = 1536        # [64,256] w1^T duplicated on partitions 0-63 and 64-127
C_W3X = 1792        # w3s^T likewise
C_WOUT = 2048       # [128, 2] wout^T k-halves
C_BC = 2050         # [128, 10] biases: col 2j+m = bias_j[128m:128(m+1)]
C_TOT = 2060

TRACE = False
TRACE_KW = {}
LAST = None
_CACHE = {}


def _build():
    import concourse.bacc as bacc
    import concourse.mybir as mybir
    import concourse.tile as tile

    F32 = mybir.dt.float32
    F32R = mybir.dt.float32r
    AF = mybir.ActivationFunctionType
    MUL = mybir.AluOpType.mult
    ADD = mybir.AluOpType.add

    nc = bacc.Bacc("TRN2", target_bir_lowering=False, debug=False, num_devices=N_CORES)

    d_xx = nc.dram_tensor("xx", [P, N], F32R, kind="ExternalInput").ap()
    d_bigw = nc.dram_tensor("bigw", [P, C_TOT], F32R, kind="ExternalInput").ap()

    d_y = nc.dram_tensor("ydram", [P, 2 * N], F32R, kind="ExternalOutput").ap()
    d_b = nc.dram_tensor("bdram", [1, N], F32, kind="ExternalOutput").ap()
    d_bd = nc.dram_tensor("bddram", [1, N], F32, kind="ExternalOutput").ap()

    MSL = [slice(0, P), slice(P, 2 * P)]
    FSL = [slice(0, N), slice(N, 2 * N)]

    with tile.TileContext(nc) as tc:
        with (
            tc.tile_pool(name="io", bufs=1) as io,
            tc.tile_pool(name="act", bufs=1) as act,
            tc.tile_pool(name="psum", bufs=4, space="PSUM") as pp,
        ):
            t_xx = io.tile([P, N], F32R, name="t_xx")
            t_bigw = io.tile([P, C_TOT], F32R, name="t_bigw")
            t_junk = act.tile([P, N], F32, name="t_junk")
            # spread input DMA issues across the two HWDGE queues (SP + ACT)
            nc.sync.dma_start(out=t_xx[:], in_=d_xx[:])
            # warmup fodder: any bytes will do; reuse the weight blob
            nc.scalar.dma_start(out=t_junk[:].bitcast(F32R), in_=d_bigw[:, 0:N])
            # early block: layer-1/3 skip weights + wout + biases (~134KB)
            nc.scalar.dma_start(out=t_bigw[:, C_W1X:C_TOT], in_=d_bigw[:, C_W1X:C_TOT])
            # late weights, one DMA each in order of first use, queues
            # alternated so transfers overlap
            nc.sync.dma_start(out=t_bigw[:, C_W2AT:C_W2BT], in_=d_bigw[:, C_W2AT:C_W2BT])
            nc.scalar.dma_start(out=t_bigw[:, C_W2BT:C_W3AT], in_=d_bigw[:, C_W2BT:C_W3AT])
            nc.sync.dma_start(out=t_bigw[:, C_W3AT:C_W1X], in_=d_bigw[:, C_W3AT:C_W1X])

            def wsl(base, lo, hi, cols):
                return t_bigw[lo:hi, base + cols.start:base + cols.stop]

            def bcol(j, m):
                c = C_BC + 2 * j + m
                return t_bigw[:, c:c + 1].bitcast(F32)

            # PE warmup: fp32 dummy matmuls (f32r work does not reliably
            # lift the HAM clock gate, fp32 does) on the junk tile while the
            # weight DMAs are in flight -> warm clock for the real work.
            p_warm = pp.tile([P, N], F32, name="p_warm", tag="ps")
            nc.tensor.matmul(p_warm[:], t_junk[:, 0:P], t_junk[:],
                             start=True, stop=True)
            nc.tensor.matmul(p_warm[:, 0:H], t_junk[:, 0:P], t_junk[:, 0:H],
                             start=True, stop=True)

            # ACT table preload: tiny activation off the const AP so the
            # 1.3us ACT_TABLE_LOAD happens during the input DMA, not on the
            # critical path of the first real eviction.
            t_warm = act.tile([1, 1], F32, name="t_warm")
            nc.scalar.activation(t_warm[:], nc.const_aps.aps[(F32, 0.0)][0:1, 0:1],
                                 AF.Square, bias=0.0, scale=1.0)

            # ---- phase 1: K=64 row-packed matmuls ----
            p_z = pp.tile([P, 2 * N], F32, name="p_z", tag="ps")
            p_u1 = pp.tile([P, 2 * N], F32, name="p_u1", tag="ps")
            p_z23 = pp.tile([P, 2 * N], F32, name="p_z23", tag="ps")
            for m in range(2):
                nc.tensor.matmul(p_z[:, FSL[m]], wsl(C_W1X, 0, NOBS, MSL[m]),
                                 t_xx[0:NOBS, :], start=True, stop=True)
                nc.tensor.matmul(p_u1[:, FSL[m]], wsl(C_W1X, NOBS, P, MSL[m]),
                                 t_xx[NOBS:P, :], start=True, stop=True)
            for m in range(2):
                nc.tensor.matmul(p_z23[:, FSL[m]], wsl(C_W3X, 0, NOBS, MSL[m]),
                                 t_xx[0:NOBS, :], start=True, stop=True)

            t_y1 = act.tile([P, 2 * N], F32R, name="t_y1")
            t_ze = act.tile([P, 2 * N], F32, name="t_ze")
            t_z23e = act.tile([P, 2 * N], F32, name="t_z23e")
            for m in range(2):
                nc.scalar.activation(t_y1[:, FSL[m]], p_z[:, FSL[m]], AF.Square,
                                     bias=bcol(0, m), scale=1.0)
            t_v1 = act.tile([P, 2 * N], F32R, name="t_v1")
            for m in range(2):
                nc.vector.tensor_scalar(t_ze[:, FSL[m]], p_z[:, FSL[m]],
                                        bcol(0, m), None, op0=ADD)
                nc.vector.scalar_tensor_tensor(t_v1[:, FSL[m]], t_ze[:, FSL[m]],
                                               2.0, p_u1[:, FSL[m]],
                                               op0=MUL, op1=MUL)

            # ---- phase 2: layer 2 ----
            p_z12 = pp.tile([P, 2 * N], F32, name="p_z12", tag="ps")
            p_z22 = pp.tile([P, 2 * N], F32, name="p_z22", tag="ps")
            p_a = pp.tile([P, 2 * N], F32, name="p_a", tag="ps")
            p_b = pp.tile([P, 2 * N], F32, name="p_b", tag="ps")
            for m in range(2):
                nc.tensor.matmul(p_z12[:, FSL[m]], wsl(C_W2AT, 0, P, MSL[m]),
                                 t_y1[:, FSL[0]], start=True, stop=False)
                nc.tensor.matmul(p_z12[:, FSL[m]],
                                 wsl(C_W2AT, 0, P, slice(H + MSL[m].start, H + MSL[m].stop)),
                                 t_y1[:, FSL[1]], start=False, stop=True)
                nc.tensor.matmul(p_z22[:, FSL[m]], wsl(C_W2BT, 0, P, MSL[m]),
                                 t_y1[:, FSL[0]], start=True, stop=False)
                nc.tensor.matmul(p_z22[:, FSL[m]],
                                 wsl(C_W2BT, 0, P, slice(H + MSL[m].start, H + MSL[m].stop)),
                                 t_y1[:, FSL[1]], start=False, stop=True)
            for m in range(2):
                nc.tensor.matmul(p_a[:, FSL[m]], wsl(C_W2BT, 0, P, MSL[m]),
                                 t_v1[:, FSL[0]], start=True, stop=False)
                nc.tensor.matmul(p_a[:, FSL[m]],
                                 wsl(C_W2BT, 0, P, slice(H + MSL[m].start, H + MSL[m].stop)),
                                 t_v1[:, FSL[1]], start=False, stop=True)
                nc.tensor.matmul(p_b[:, FSL[m]], wsl(C_W2AT, 0, P, MSL[m]),
                                 t_v1[:, FSL[0]], start=True, stop=False)
                nc.tensor.matmul(p_b[:, FSL[m]],
                                 wsl(C_W2AT, 0, P, slice(H + MSL[m].start, H + MSL[m].stop)),
                                 t_v1[:, FSL[1]], start=False, stop=True)

            t_z12e = act.tile([P, 2 * N], F32, name="t_z12e")
            t_z22e = act.tile([P, 2 * N], F32, name="t_z22e")
            for m in range(2):
                nc.scalar.activation(t_z12e[:, FSL[m]], p_z12[:, FSL[m]], AF.Identity,
                                     bias=bcol(1, m), scale=1.0)
                nc.vector.tensor_scalar(t_z22e[:, FSL[m]], p_z22[:, FSL[m]],
                                        bcol(2, m), None, op0=ADD)
            for m in range(2):
                nc.scalar.activation(t_z23e[:, FSL[m]], p_z23[:, FSL[m]], AF.Identity,
                                     bias=bcol(4, m), scale=1.0)
            t_y2 = act.tile([P, 2 * N], F32R, name="t_y2")
            t_t1 = act.tile([P, 2 * N], F32R, name="t_t1")
            t_t2 = act.tile([P, 2 * N], F32R, name="t_t2")
            for m in range(2):
                nc.vector.tensor_mul(t_t1[:, FSL[m]], t_z12e[:, FSL[m]], p_a[:, FSL[m]])
            for m in range(2):
                nc.vector.tensor_mul(t_t2[:, FSL[m]], t_z22e[:, FSL[m]], p_b[:, FSL[m]])
            for m in range(2):
                nc.gpsimd.tensor_mul(t_y2[:, FSL[m]], t_z12e[:, FSL[m]], t_z22e[:, FSL[m]])

            # ---- phase 3: layer 3 (u3s joins here to ease PSUM pressure) ----
            p_z13 = pp.tile([P, 2 * N], F32, name="p_z13", tag="ps")
            p_g = pp.tile([P, 2 * N], F32, name="p_g", tag="ps")
            p_u3s = pp.tile([P, 2 * N], F32, name="p_u3s", tag="ps")
            for m in range(2):
                nc.tensor.matmul(p_u3s[:, FSL[m]], wsl(C_W3X, NOBS, P, MSL[m]),
                                 t_xx[NOBS:P, :], start=True, stop=True)
            for m in range(2):
                nc.tensor.matmul(p_z13[:, FSL[m]], wsl(C_W3AT, 0, P, MSL[m]),
                                 t_y2[:, FSL[0]], start=True, stop=False)
                nc.tensor.matmul(p_z13[:, FSL[m]],
                                 wsl(C_W3AT, 0, P, slice(H + MSL[m].start, H + MSL[m].stop)),
                                 t_y2[:, FSL[1]], start=False, stop=True)
                nc.tensor.matmul(p_g[:, FSL[m]], wsl(C_W3AT, 0, P, MSL[m]),
                                 t_t1[:, FSL[0]], start=True, stop=False)
                nc.tensor.matmul(p_g[:, FSL[m]],
                                 wsl(C_W3AT, 0, P, slice(H + MSL[m].start, H + MSL[m].stop)),
                                 t_t1[:, FSL[1]], start=False, stop=False)
                nc.tensor.matmul(p_g[:, FSL[m]], wsl(C_W3AT, 0, P, MSL[m]),
                                 t_t2[:, FSL[0]], start=False, stop=False)
                nc.tensor.matmul(p_g[:, FSL[m]],
                                 wsl(C_W3AT, 0, P, slice(H + MSL[m].start, H + MSL[m].stop)),
                                 t_t2[:, FSL[1]], start=False, stop=True)

            t_z13e = act.tile([P, 2 * N], F32, name="t_z13e")
            for m in range(2):
                nc.scalar.activation(t_z13e[:, FSL[m]], p_z13[:, FSL[m]], AF.Identity,
                                     bias=bcol(3, m), scale=1.0)
            t_y3 = act.tile([P, 2 * N], F32R, name="t_y3")
            t_t3 = act.tile([P, 2 * N], F32R, name="t_t3")
            t_t4 = act.tile([P, 2 * N], F32R, name="t_t4")
            for m in range(2):
                nc.vector.tensor_mul(t_t3[:, FSL[m]], t_z13e[:, FSL[m]], p_u3s[:, FSL[m]])
            for m in range(2):
                nc.gpsimd.tensor_mul(t_y3[:, FSL[m]], t_z13e[:, FSL[m]], t_z23e[:, FSL[m]])
            for m in range(2):
                nc.vector.tensor_mul(t_t4[:, FSL[m]], t_z23e[:, FSL[m]], p_g[:, FSL[m]])
            nc.sync.dma_start(out=d_y[:], in_=t_y3[:])

            # ---- phase 4: wout contractions (M=1) ----
            p_bout = pp.tile([1, N], F32, name="p_bout", tag="ps")
            p_bd = pp.tile([1, N], F32, name="p_bd", tag="ps")
            wo = [t_bigw[:, C_WOUT + k:C_WOUT + k + 1] for k in range(2)]
            nc.tensor.matmul(p_bd[0:1, :], wo[0], t_t3[:, FSL[0]], start=True, stop=False)
            nc.tensor.matmul(p_bd[0:1, :], wo[1], t_t3[:, FSL[1]], start=False, stop=False)
            nc.tensor.matmul(p_bout[0:1, :], wo[0], t_y3[:, FSL[0]], start=True, stop=False)
            nc.tensor.matmul(p_bout[0:1, :], wo[1], t_y3[:, FSL[1]], start=False, stop=True)
            nc.tensor.matmul(p_bd[0:1, :], wo[0], t_t4[:, FSL[0]], start=False, stop=False)
            nc.tensor.matmul(p_bd[0:1, :], wo[1], t_t4[:, FSL[1]], start=False, stop=True)

            t_brow = act.tile([1, N], F32, name="t_brow")
            t_bdrow = act.tile([1, N], F32, name="t_bdrow")
            nc.scalar.copy(t_brow[:], p_bout[0:1, :])
            nc.vector.tensor_copy(t_bdrow[:], p_bd[0:1, :])
            nc.sync.dma_start(out=d_b[:], in_=t_brow[:])
            nc.sync.dma_start(out=d_bd[:], in_=t_bdrow[:])

    nc.compile()
    return nc


def kernel(x, xdot, w1, b1, w2a, b2a, w2b, b2b, w3a, b3a, w3s, b3s, wout, scalar):
    from concourse.bass_utils import run_bass_kernel_spmd

    global LAST
    if "nc" not in _CACHE:
        _CACHE["nc"] = _build()
    nc = _CACHE["nc"]

    f = np.float32
    x = np.asarray(x, f)
    xdot = np.asarray(xdot, f)
    sval = np.asarray(scalar, f).reshape(-1)[0]

    xt_full = np.ascontiguousarray(x.T)
    xdt_full = np.ascontiguousarray(xdot.T)

    def ksplit(w):  # [H, H] -> [128, 512] (k0 | k1)
        wt = np.asarray(w, f).T
        return np.concatenate([wt[:P], wt[P:]], axis=1)

    bigw = np.zeros((P, C_TOT), f)
    bigw[:, C_W2AT:C_W2AT + 2 * H] = ksplit(w2a)
    bigw[:, C_W2BT:C_W2BT + 2 * H] = ksplit(w2b)
    bigw[:, C_W3AT:C_W3AT + 2 * H] = ksplit(w3a)
    w1t = np.asarray(w1, f).T
    w3st = np.asarray(w3s, f).T
    bigw[:NOBS, C_W1X:C_W1X + H] = w1t
    bigw[NOBS:, C_W1X:C_W1X + H] = w1t
    bigw[:NOBS, C_W3X:C_W3X + H] = w3st
    bigw[NOBS:, C_W3X:C_W3X + H] = w3st
    wo = np.asarray(wout, f)[0]
    bigw[:, C_WOUT] = wo[:P]
    bigw[:, C_WOUT + 1] = wo[P:]
    for j, bias in enumerate((b1, b2a, b2b, b3a, b3s)):
        bb = np.asarray(bias, f)
        bigw[:, C_BC + 2 * j] = bb[:P]
        bigw[:, C_BC + 2 * j + 1] = bb[P:]
    bigw = np.ascontiguousarray(bigw)

    in_maps = []
    for c in range(N_CORES):
        sl = slice(c * BL, (c + 1) * BL)
        xx = np.empty((P, BL), f)
        xx[:NOBS] = xt_full[:, sl]
        xx[NOBS:] = xdt_full[:, sl]
        in_maps.append({"xx": xx, "bigw": bigw})

    res = run_bass_kernel_spmd(
        nc, in_maps, core_ids=list(range(N_CORES)),
        trace=TRACE, **TRACE_KW)
    LAST = res

    yb = np.empty((B, 1), f)
    ybdot = np.empty((B,), f)
    yfull = np.empty((B, H), f)
    for c in range(N_CORES):
        sl = slice(c * BL, (c + 1) * BL)
        r = res.results[c]
        yb[sl, 0] = r["bdram"][0]
        ybdot[sl] = r["bddram"][0]
        yfull[sl, :P] = r["ydram"][:, :BL].T
        yfull[sl, P:] = r["ydram"][:, BL:].T
    yyfull = np.broadcast_to(np.float32(sval), (B, NOBS)).copy()
    yyfull += x * 0
    return yb, ybdot, yfull, yyfull


# revision 20
# speedup vs baseline: 1.1469x; 1.0732x over previous
"""Trainium2 Bass kernel for nn_Net_25254407701209 (dense_mlp).

Reference math (B=4096, N_OBS=64, H=256):
    z   = x w1^T + b1;  y1 = z^2
    z12 = y1 w2a^T + b2a; z22 = y1 w2b^T + b2b; y2 = z12*z22
    z13 = y2 w3a^T + b3a; z23 = x w3s^T + b3s;  y3 = z13*z23
    b   = y3 wout^T;  yy = scalar broadcast
The reference's full-Jacobian chain collapses to a forward-mode JVP with
tangent xdot:
    v1 = 2 z * (xdot w1^T)
    t1 = z12 * (v1 w2b^T);  t2 = z22 * (v1 w2a^T)      (v2 = t1 + t2)
    g  = w3a v2 = w3a t1 + w3a t2     <- add folded into PSUM accumulation
    t3 = z13 * (xdot w3s^T);  t4 = z23 * g             (v3 = t3 + t4)
    bdot = wout . v3 = wout.t3 + wout.t4               <- folded likewise

Sharding: pure data-parallel, batch 4096 -> 8 cores x 512 rows. Feature-on-
partition layout: each [512, 256] activation is one [128, 1024] tile (free
block m = features 128m..128m+127). Matmuls in float32r (1 cycle/row).
All weights arrive in ONE packed DMA (BIGW), x/xdot in another (XX, with
x on partitions 0-63 and xdot on 64-127 so the K=64 layer-1 matmuls
row-pack into concurrent array halves). Biases fold into PSUM->SBUF
evictions. yy is an input-independent broadcast, assembled host-side.
"""
import sys

if "/opt/trn_rl_repo" not in sys.path:
    sys.path.insert(0, "/opt/trn_rl_repo")

import numpy as np

N_CORES = 8
B, NOBS, H = 4096, 64, 256
BL = B // N_CORES
P = 128
N = BL

# BIGW column layout
C_W2AT = 0          # [128, 512] w2a^T (k0 | k1)
C_W2BT = 512
C_W3AT = 1024
C_W1X = 1536        # [64,256] w1^T duplicated on partitions 0-63 and 64-127
C_W3X = 1792        # w3s^T likewise
C_WOUT = 2048       # [128, 2] wout^T k-halves
C_BC = 2050         # [128, 10] biases: col 2j+m = bias_j[128m:128(m+1)]
C_TOT = 2060

TRACE = False
TRACE_KW = {}
LAST = None
_CACHE = {}


def _build():
    import concourse.bacc as bacc
    import concourse.mybir as mybir
    import concourse.tile as tile

    F32 = mybir.dt.float32
    F32R = mybir.dt.float32r
    AF = mybir.ActivationFunctionType
    MUL = mybir.AluOpType.mult
    ADD = mybir.AluOpType.add

    nc = bacc.Bacc("TRN2", target_bir_lowering=False, debug=False, num_devices=N_CORES)

    d_xx = nc.dram_tensor("xx", [P, N], F32R, kind="ExternalInput").ap()
    d_bigw = nc.dram_tensor("bigw", [P, C_TOT], F32R, kind="ExternalInput").ap()

    d_y = nc.dram_tensor("ydram", [P, 2 * N], F32R, kind="ExternalOutput").ap()
    d_b = nc.dram_tensor("bdram", [1, N], F32, kind="ExternalOutput").ap()
    d_bd = nc.dram_tensor("bddram", [1, N], F32, kind="ExternalOutput").ap()

    MSL = [slice(0, P), slice(P, 2 * P)]
    FSL = [slice(0, N), slice(N, 2 * N)]

    with tile.TileContext(nc) as tc:
        with (
            tc.tile_pool(name="io", bufs=1) as io,
            tc.tile_pool(name="act", bufs=1) as act,
            tc.tile_pool(name="psum", bufs=4, space="PSUM") as pp,
        ):
            t_xx = io.tile([P, N], F32R, name="t_xx")
            t_bigw = io.tile([P, C_TOT], F32R, name="t_bigw")
            t_junk = act.tile([P, N], F32, name="t_junk")
            # warmup fodder written on-chip (gpsimd is free early); a DMA'd
            # junk tile completes LAST behind the real inputs on the shared
            # HW queues and would delay the warmup by ~3us
            nc.gpsimd.memset(t_junk[:], 0.0)
            # spread input DMA issues across the two HWDGE queues (SP + ACT)
            nc.sync.dma_start(out=t_xx[:], in_=d_xx[:])
            # early block: layer-1/3 skip weights + wout + biases (~134KB)
            nc.scalar.dma_start(out=t_bigw[:, C_W1X:C_TOT], in_=d_bigw[:, C_W1X:C_TOT])
            # late weights, one DMA each in order of first use, queues
            # alternated so transfers overlap
            nc.sync.dma_start(out=t_bigw[:, C_W2AT:C_W2BT], in_=d_bigw[:, C_W2AT:C_W2BT])
            nc.scalar.dma_start(out=t_bigw[:, C_W2BT:C_W3AT], in_=d_bigw[:, C_W2BT:C_W3AT])
            nc.sync.dma_start(out=t_bigw[:, C_W3AT:C_W1X], in_=d_bigw[:, C_W3AT:C_W1X])

            def wsl(base, lo, hi, cols):
                return t_bigw[lo:hi, base + cols.start:base + cols.stop]

            def bcol(j, m):
                c = C_BC + 2 * j + m
                return t_bigw[:, c:c + 1].bitcast(F32)

            # PE warmup: fp32 dummy matmuls (f32r work does not reliably
            # lift the HAM clock gate, fp32 does) on the junk tile while the
            # weight DMAs are in flight -> warm clock for the real work.
            p_warm = pp.tile([P, N], F32, name="p_warm", tag="ps")
            nc.tensor.matmul(p_warm[:], t_junk[:, 0:P], t_junk[:],
                             start=True, stop=True)
            nc.tensor.matmul(p_warm[:, 0:H], t_junk[:, 0:P], t_junk[:, 0:H],
                             start=True, stop=True)

            # ACT table preload: tiny activation off the const AP so the
            # 1.3us ACT_TABLE_LOAD happens during the input DMA, not on the
            # critical path of the first real eviction.
            t_warm = act.tile([1, 1], F32, name="t_warm")
            nc.scalar.activation(t_warm[:], nc.const_aps.aps[(F32, 0.0)][0:1, 0:1],
                                 AF.Square, bias=0.0, scale=1.0)

            # ---- phase 1: K=64 row-packed matmuls ----
            p_z = pp.tile([P, 2 * N], F32, name="p_z", tag="ps")
            p_u1 = pp.tile([P, 2 * N], F32, name="p_u1", tag="ps")
            p_z23 = pp.tile([P, 2 * N], F32, name="p_z23", tag="ps")
            for m in range(2):
                nc.tensor.matmul(p_z[:, FSL[m]], wsl(C_W1X, 0, NOBS, MSL[m]),
                                 t_xx[0:NOBS, :], start=True, stop=True)
                nc.tensor.matmul(p_u1[:, FSL[m]], wsl(C_W1X, NOBS, P, MSL[m]),
                                 t_xx[NOBS:P, :], start=True, stop=True)
            for m in range(2):
                nc.tensor.matmul(p_z23[:, FSL[m]], wsl(C_W3X, 0, NOBS, MSL[m]),
                                 t_xx[0:NOBS, :], start=True, stop=True)

            t_y1 = act.tile([P, 2 * N], F32R, name="t_y1")
            t_ze = act.tile([P, 2 * N], F32, name="t_ze")
            t_z23e = act.tile([P, 2 * N], F32, name="t_z23e")
            for m in range(2):
                nc.scalar.activation(t_y1[:, FSL[m]], p_z[:, FSL[m]], AF.Square,
                                     bias=bcol(0, m), scale=1.0)
            t_v1 = act.tile([P, 2 * N], F32R, name="t_v1")
            for m in range(2):
                nc.vector.tensor_scalar(t_ze[:, FSL[m]], p_z[:, FSL[m]],
                                        bcol(0, m), None, op0=ADD)
                nc.vector.scalar_tensor_tensor(t_v1[:, FSL[m]], t_ze[:, FSL[m]],
                                               2.0, p_u1[:, FSL[m]],
                                               op0=MUL, op1=MUL)

            # ---- phase 2: layer 2 ----
            p_z12 = pp.tile([P, 2 * N], F32, name="p_z12", tag="ps")
            p_z22 = pp.tile([P, 2 * N], F32, name="p_z22", tag="ps")
            p_a = pp.tile([P, 2 * N], F32, name="p_a", tag="ps")
            p_b = pp.tile([P, 2 * N], F32, name="p_b", tag="ps")
            for m in range(2):
                nc.tensor.matmul(p_z12[:, FSL[m]], wsl(C_W2AT, 0, P, MSL[m]),
                                 t_y1[:, FSL[0]], start=True, stop=False)
                nc.tensor.matmul(p_z12[:, FSL[m]],
                                 wsl(C_W2AT, 0, P, slice(H + MSL[m].start, H + MSL[m].stop)),
                                 t_y1[:, FSL[1]], start=False, stop=True)
                nc.tensor.matmul(p_z22[:, FSL[m]], wsl(C_W2BT, 0, P, MSL[m]),
                                 t_y1[:, FSL[0]], start=True, stop=False)
                nc.tensor.matmul(p_z22[:, FSL[m]],
                                 wsl(C_W2BT, 0, P, slice(H + MSL[m].start, H + MSL[m].stop)),
                                 t_y1[:, FSL[1]], start=False, stop=True)
            for m in range(2):
                nc.tensor.matmul(p_a[:, FSL[m]], wsl(C_W2BT, 0, P, MSL[m]),
                                 t_v1[:, FSL[0]], start=True, stop=False)
                nc.tensor.matmul(p_a[:, FSL[m]],
                                 wsl(C_W2BT, 0, P, slice(H + MSL[m].start, H + MSL[m].stop)),
                                 t_v1[:, FSL[1]], start=False, stop=True)
                nc.tensor.matmul(p_b[:, FSL[m]], wsl(C_W2AT, 0, P, MSL[m]),
                                 t_v1[:, FSL[0]], start=True, stop=False)
                nc.tensor.matmul(p_b[:, FSL[m]],
                                 wsl(C_W2AT, 0, P, slice(H + MSL[m].start, H + MSL[m].stop)),
                                 t_v1[:, FSL[1]], start=False, stop=True)

            t_z12e = act.tile([P, 2 * N], F32, name="t_z12e")
            t_z22e = act.tile([P, 2 * N], F32, name="t_z22e")
            for m in range(2):
                nc.scalar.activation(t_z12e[:, FSL[m]], p_z12[:, FSL[m]], AF.Identity,
                                     bias=bcol(1, m), scale=1.0)
                nc.vector.tensor_scalar(t_z22e[:, FSL[m]], p_z22[:, FSL[m]],
                                        bcol(2, m), None, op0=ADD)
            for m in range(2):
                nc.scalar.activation(t_z23e[:, FSL[m]], p_z23[:, FSL[m]], AF.Identity,
                                     bias=bcol(4, m), scale=1.0)
            t_y2 = act.tile([P, 2 * N], F32R, name="t_y2")
            t_t1 = act.tile([P, 2 * N], F32R, name="t_t1")
            t_t2 = act.tile([P, 2 * N], F32R, name="t_t2")
            for m in range(2):
                nc.vector.tensor_mul(t_t1[:, FSL[m]], t_z12e[:, FSL[m]], p_a[:, FSL[m]])
            for m in range(2):
                nc.vector.tensor_mul(t_t2[:, FSL[m]], t_z22e[:, FSL[m]], p_b[:, FSL[m]])
            for m in range(2):
                nc.gpsimd.tensor_mul(t_y2[:, FSL[m]], t_z12e[:, FSL[m]], t_z22e[:, FSL[m]])

            # ---- phase 3: layer 3 (u3s joins here to ease PSUM pressure) ----
            p_z13 = pp.tile([P, 2 * N], F32, name="p_z13", tag="ps")
            p_g = pp.tile([P, 2 * N], F32, name="p_g", tag="ps")
            p_u3s = pp.tile([P, 2 * N], F32, name="p_u3s", tag="ps")
            for m in range(2):
                nc.tensor.matmul(p_u3s[:, FSL[m]], wsl(C_W3X, NOBS, P, MSL[m]),
                                 t_xx[NOBS:P, :], start=True, stop=True)
            for m in range(2):
                nc.tensor.matmul(p_z13[:, FSL[m]], wsl(C_W3AT, 0, P, MSL[m]),
                                 t_y2[:, FSL[0]], start=True, stop=False)
                nc.tensor.matmul(p_z13[:, FSL[m]],
                                 wsl(C_W3AT, 0, P, slice(H + MSL[m].start, H + MSL[m].stop)),
                                 t_y2[:, FSL[1]], start=False, stop=True)
                nc.tensor.matmul(p_g[:, FSL[m]], wsl(C_W3AT, 0, P, MSL[m]),
                                 t_t1[:, FSL[0]], start=True, stop=False)
                nc.tensor.matmul(p_g[:, FSL[m]],
                                 wsl(C_W3AT, 0, P, slice(H + MSL[m].start, H + MSL[m].stop)),
                                 t_t1[:, FSL[1]], start=False, stop=False)
                nc.tensor.matmul(p_g[:, FSL[m]], wsl(C_W3AT, 0, P, MSL[m]),
                                 t_t2[:, FSL[0]], start=False, stop=False)
                nc.tensor.matmul(p_g[:, FSL[m]],
                                 wsl(C_W3AT, 0, P, slice(H + MSL[m].start, H + MSL[m].stop)),
                                 t_t2[:, FSL[1]], start=False, stop=True)

            t_z13e = act.tile([P, 2 * N], F32, name="t_z13e")
            for m in range(2):
                nc.scalar.activation(t_z13e[:, FSL[m]], p_z13[:, FSL[m]], AF.Identity,
                                     bias=bcol(3, m), scale=1.0)
            t_y3 = act.tile([P, 2 * N], F32R, name="t_y3")
            t_t3 = act.tile([P, 2 * N], F32R, name="t_t3")
            t_t4 = act.tile([P, 2 * N], F32R, name="t_t4")
            for m in range(2):
                nc.vector.tensor_mul(t_t3[:, FSL[m]], t_z13e[:, FSL[m]], p_u3s[:, FSL[m]])
            for m in range(2):
                nc.gpsimd.tensor_mul(t_y3[:, FSL[m]], t_z13e[:, FSL[m]], t_z23e[:, FSL[m]])
            for m in range(2):
                nc.vector.tensor_mul(t_t4[:, FSL[m]], t_z23e[:, FSL[m]], p_g[:, FSL[m]])
            nc.sync.dma_start(out=d_y[:], in_=t_y3[:])

            # ---- phase 4: wout contractions (M=1) ----
            p_bout = pp.tile([1, N], F32, name="p_bout", tag="ps")
            p_bd = pp.tile([1, N], F32, name="p_bd", tag="ps")
            wo = [t_bigw[:, C_WOUT + k:C_WOUT + k + 1] for k in range(2)]
            nc.tensor.matmul(p_bd[0:1, :], wo[0], t_t3[:, FSL[0]], start=True, stop=False)
            nc.tensor.matmul(p_bd[0:1, :], wo[1], t_t3[:, FSL[1]], start=False, stop=False)
            nc.tensor.matmul(p_bout[0:1, :], wo[0], t_y3[:, FSL[0]], start=True, stop=False)
            nc.tensor.matmul(p_bout[0:1, :], wo[1], t_y3[:, FSL[1]], start=False, stop=True)
            nc.tensor.matmul(p_bd[0:1, :], wo[0], t_t4[:, FSL[0]], start=False, stop=False)
            nc.tensor.matmul(p_bd[0:1, :], wo[1], t_t4[:, FSL[1]], start=False, stop=True)

            t_brow = act.tile([1, N], F32, name="t_brow")
            t_bdrow = act.tile([1, N], F32, name="t_bdrow")
            nc.scalar.copy(t_brow[:], p_bout[0:1, :])
            nc.vector.tensor_copy(t_bdrow[:], p_bd[0:1, :])
            nc.sync.dma_start(out=d_b[:], in_=t_brow[:])
            nc.sync.dma_start(out=d_bd[:], in_=t_bdrow[:])

    nc.compile()
    return nc


def kernel(x, xdot, w1, b1, w2a, b2a, w2b, b2b, w3a, b3a, w3s, b3s, wout, scalar):
    from concourse.bass_utils import run_bass_kernel_spmd

    global LAST
    if "nc" not in _CACHE:
        _CACHE["nc"] = _build()
    nc = _CACHE["nc"]

    f = np.float32
    x = np.asarray(x, f)
    xdot = np.asarray(xdot, f)
    sval = np.asarray(scalar, f).reshape(-1)[0]

    xt_full = np.ascontiguousarray(x.T)
    xdt_full = np.ascontiguousarray(xdot.T)

    def ksplit(w):  # [H, H] -> [128, 512] (k0 | k1)
        wt = np.asarray(w, f).T
        return np.concatenate([wt[:P], wt[P:]], axis=1)

    bigw = np.zeros((P, C_TOT), f)
    bigw[:, C_W2AT:C_W2AT + 2 * H] = ksplit(w2a)
    bigw[:, C_W2BT:C_W2BT + 2 * H] = ksplit(w2b)
    bigw[:, C_W3AT:C_W3AT + 2 * H] = ksplit(w3a)
    w1t = np.asarray(w1, f).T
    w3st = np.asarray(w3s, f).T
    bigw[:NOBS, C_W1X:C_W1X + H] = w1t
    bigw[NOBS:, C_W1X:C_W1X + H] = w1t
    bigw[:NOBS, C_W3X:C_W3X + H] = w3st
    bigw[NOBS:, C_W3X:C_W3X + H] = w3st
    wo = np.asarray(wout, f)[0]
    bigw[:, C_WOUT] = wo[:P]
    bigw[:, C_WOUT + 1] = wo[P:]
    for j, bias in enumerate((b1, b2a, b2b, b3a, b3s)):
        bb = np.asarray(bias, f)
        bigw[:, C_BC + 2 * j] = bb[:P]
        bigw[:, C_BC + 2 * j + 1] = bb[P:]
    bigw = np.ascontiguousarray(bigw)

    in_maps = []
    for c in range(N_CORES):
        sl = slice(c * BL, (c + 1) * BL)
        xx = np.empty((P, BL), f)
        xx[:NOBS] = xt_full[:, sl]
        xx[NOBS:] = xdt_full[:, sl]
        in_maps.append({"xx": xx, "bigw": bigw})

    res = run_bass_kernel_spmd(
        nc, in_maps, core_ids=list(range(N_CORES)),
        trace=TRACE, **TRACE_KW)
    LAST = res

    yb = np.empty((B, 1), f)
    ybdot = np.empty((B,), f)
    yfull = np.empty((B, H), f)
    for c in range(N_CORES):
        sl = slice(c * BL, (c + 1) * BL)
        r = res.results[c]
        yb[sl, 0] = r["bdram"][0]
        ybdot[sl] = r["bddram"][0]
        yfull[sl, :P] = r["ydram"][:, :BL].T
        yfull[sl, P:] = r["ydram"][:, BL:].T
    yyfull = np.broadcast_to(np.float32(sval), (B, NOBS)).copy()
    yyfull += x * 0
    return yb, ybdot, yfull, yyfull


# revision 21
# speedup vs baseline: 1.2416x; 1.0825x over previous
"""Trainium2 Bass kernel for nn_Net_25254407701209 (dense_mlp).

Reference math (B=4096, N_OBS=64, H=256):
    z   = x w1^T + b1;  y1 = z^2
    z12 = y1 w2a^T + b2a; z22 = y1 w2b^T + b2b; y2 = z12*z22
    z13 = y2 w3a^T + b3a; z23 = x w3s^T + b3s;  y3 = z13*z23
    b   = y3 wout^T;  yy = scalar broadcast
The reference's full-Jacobian chain collapses to a forward-mode JVP with
tangent xdot:
    v1 = 2 z * (xdot w1^T)
    t1 = z12 * (v1 w2b^T);  t2 = z22 * (v1 w2a^T)      (v2 = t1 + t2)
    g  = w3a v2 = w3a t1 + w3a t2     <- add folded into PSUM accumulation
    t3 = z13 * (xdot w3s^T);  t4 = z23 * g             (v3 = t3 + t4)
    bdot = wout . v3 = wout.t3 + wout.t4               <- folded likewise

Sharding: pure data-parallel, batch 4096 -> 8 cores x 512 rows. Feature-on-
partition layout: each [512, 256] activation is one [128, 1024] tile (free
block m = features 128m..128m+127). Matmuls in float32r (1 cycle/row).
All weights arrive in ONE packed DMA (BIGW), x/xdot in another (XX, with
x on partitions 0-63 and xdot on 64-127 so the K=64 layer-1 matmuls
row-pack into concurrent array halves). Biases fold into PSUM->SBUF
evictions. yy is an input-independent broadcast, assembled host-side.
"""
import sys

if "/opt/trn_rl_repo" not in sys.path:
    sys.path.insert(0, "/opt/trn_rl_repo")

import numpy as np

N_CORES = 8
B, NOBS, H = 4096, 64, 256
BL = B // N_CORES
P = 128
N = BL

# BIGW column layout
C_W2AT = 0          # [128, 512] w2a^T (k0 | k1)
C_W2BT = 512
C_W3AT = 1024
C_W1X = 1536        # [64,256] w1^T duplicated on partitions 0-63 and 64-127
C_W3X = 1792        # w3s^T likewise
C_WOUT = 2048       # [128, 2] wout^T k-halves
C_BC = 2050         # [128, 10] biases: col 2j+m = bias_j[128m:128(m+1)]
C_TOT = 2060

TRACE = False
TRACE_KW = {}
LAST = None
_CACHE = {}


def _build():
    import concourse.bacc as bacc
    import concourse.mybir as mybir
    import concourse.tile as tile

    F32 = mybir.dt.float32
    F32R = mybir.dt.float32r
    AF = mybir.ActivationFunctionType
    MUL = mybir.AluOpType.mult
    ADD = mybir.AluOpType.add

    nc = bacc.Bacc("TRN2", target_bir_lowering=False, debug=False, num_devices=N_CORES)

    d_xx = nc.dram_tensor("xx", [P, N], F32R, kind="ExternalInput").ap()
    d_bigw = nc.dram_tensor("bigw", [P, C_TOT], F32R, kind="ExternalInput").ap()

    d_y = nc.dram_tensor("ydram", [P, 2 * N], F32R, kind="ExternalOutput").ap()
    d_b = nc.dram_tensor("bdram", [1, N], F32, kind="ExternalOutput").ap()
    d_bd = nc.dram_tensor("bddram", [1, N], F32, kind="ExternalOutput").ap()

    MSL = [slice(0, P), slice(P, 2 * P)]
    FSL = [slice(0, N), slice(N, 2 * N)]

    with tile.TileContext(nc) as tc:
        with (
            tc.tile_pool(name="io", bufs=1) as io,
            tc.tile_pool(name="act", bufs=1) as act,
            tc.tile_pool(name="psum", bufs=4, space="PSUM") as pp,
        ):
            t_xx = io.tile([P, N], F32R, name="t_xx")
            t_bigw = io.tile([P, C_TOT], F32R, name="t_bigw")
            t_junk = act.tile([P, N], F32, name="t_junk")
            # warmup fodder written on-chip (gpsimd is free early); a DMA'd
            # junk tile completes LAST behind the real inputs on the shared
            # HW queues and would delay the warmup by ~3us
            nc.gpsimd.memset(t_junk[:], 0.0)
            # spread input DMA issues across the two HWDGE queues (SP + ACT)
            nc.sync.dma_start(out=t_xx[:], in_=d_xx[:])
            # early block: layer-1/3 skip weights + wout + biases (~134KB)
            nc.scalar.dma_start(out=t_bigw[:, C_W1X:C_TOT], in_=d_bigw[:, C_W1X:C_TOT])
            # late weights, one DMA each in order of first use, queues
            # alternated so transfers overlap
            nc.sync.dma_start(out=t_bigw[:, C_W2AT:C_W2BT], in_=d_bigw[:, C_W2AT:C_W2BT])
            nc.scalar.dma_start(out=t_bigw[:, C_W2BT:C_W3AT], in_=d_bigw[:, C_W2BT:C_W3AT])
            nc.sync.dma_start(out=t_bigw[:, C_W3AT:C_W1X], in_=d_bigw[:, C_W3AT:C_W1X])

            def wsl(base, lo, hi, cols):
                return t_bigw[lo:hi, base + cols.start:base + cols.stop]

            def bcol(j, m):
                c = C_BC + 2 * j + m
                return t_bigw[:, c:c + 1].bitcast(F32)

            # PE warmup: fp32 dummy matmuls (f32r work does not reliably
            # lift the HAM clock gate, fp32 does) on the junk tile while the
            # weight DMAs are in flight -> warm clock for the real work.
            p_warm = pp.tile([P, N], F32, name="p_warm", tag="ps")
            nc.tensor.matmul(p_warm[:], t_junk[:, 0:P], t_junk[:],
                             start=True, stop=True)
            nc.tensor.matmul(p_warm[:, 0:H], t_junk[:, 0:P], t_junk[:, 0:H],
                             start=True, stop=True)
            # third short warmup fills the PE idle slot while the xx DMA
            # lands AND pushes fp32 busy past a full free-running HAM
            # window, so the clock reliably flips before the real matmuls
            nc.tensor.matmul(p_warm[:, 0:H], t_junk[:, 0:P], t_junk[:, H:2 * H],
                             start=True, stop=True)

            # ACT table preload: tiny activation off the const AP so the
            # 1.3us ACT_TABLE_LOAD happens during the input DMA, not on the
            # critical path of the first real eviction.
            t_warm = act.tile([1, 1], F32, name="t_warm")
            nc.scalar.activation(t_warm[:], nc.const_aps.aps[(F32, 0.0)][0:1, 0:1],
                                 AF.Square, bias=0.0, scale=1.0)

            # ---- phase 1: K=64 row-packed matmuls ----
            p_z = pp.tile([P, 2 * N], F32, name="p_z", tag="ps")
            p_u1 = pp.tile([P, 2 * N], F32, name="p_u1", tag="ps")
            p_z23 = pp.tile([P, 2 * N], F32, name="p_z23", tag="ps")
            for m in range(2):
                nc.tensor.matmul(p_z[:, FSL[m]], wsl(C_W1X, 0, NOBS, MSL[m]),
                                 t_xx[0:NOBS, :], start=True, stop=True)
                nc.tensor.matmul(p_u1[:, FSL[m]], wsl(C_W1X, NOBS, P, MSL[m]),
                                 t_xx[NOBS:P, :], start=True, stop=True)
            for m in range(2):
                nc.tensor.matmul(p_z23[:, FSL[m]], wsl(C_W3X, 0, NOBS, MSL[m]),
                                 t_xx[0:NOBS, :], start=True, stop=True)

            t_y1 = act.tile([P, 2 * N], F32R, name="t_y1")
            t_ze = act.tile([P, 2 * N], F32, name="t_ze")
            t_z23e = act.tile([P, 2 * N], F32, name="t_z23e")
            for m in range(2):
                nc.scalar.activation(t_y1[:, FSL[m]], p_z[:, FSL[m]], AF.Square,
                                     bias=bcol(0, m), scale=1.0)
            t_v1 = act.tile([P, 2 * N], F32R, name="t_v1")
            for m in range(2):
                nc.vector.tensor_scalar(t_ze[:, FSL[m]], p_z[:, FSL[m]],
                                        bcol(0, m), None, op0=ADD)
                nc.vector.scalar_tensor_tensor(t_v1[:, FSL[m]], t_ze[:, FSL[m]],
                                               2.0, p_u1[:, FSL[m]],
                                               op0=MUL, op1=MUL)

            # ---- phase 2: layer 2 ----
            p_z12 = pp.tile([P, 2 * N], F32, name="p_z12", tag="ps")
            p_z22 = pp.tile([P, 2 * N], F32, name="p_z22", tag="ps")
            p_a = pp.tile([P, 2 * N], F32, name="p_a", tag="ps")
            p_b = pp.tile([P, 2 * N], F32, name="p_b", tag="ps")
            for m in range(2):
                nc.tensor.matmul(p_z12[:, FSL[m]], wsl(C_W2AT, 0, P, MSL[m]),
                                 t_y1[:, FSL[0]], start=True, stop=False)
                nc.tensor.matmul(p_z12[:, FSL[m]],
                                 wsl(C_W2AT, 0, P, slice(H + MSL[m].start, H + MSL[m].stop)),
                                 t_y1[:, FSL[1]], start=False, stop=True)
                nc.tensor.matmul(p_z22[:, FSL[m]], wsl(C_W2BT, 0, P, MSL[m]),
                                 t_y1[:, FSL[0]], start=True, stop=False)
                nc.tensor.matmul(p_z22[:, FSL[m]],
                                 wsl(C_W2BT, 0, P, slice(H + MSL[m].start, H + MSL[m].stop)),
                                 t_y1[:, FSL[1]], start=False, stop=True)
            for m in range(2):
                nc.tensor.matmul(p_a[:, FSL[m]], wsl(C_W2BT, 0, P, MSL[m]),
                                 t_v1[:, FSL[0]], start=True, stop=False)
                nc.tensor.matmul(p_a[:, FSL[m]],
                                 wsl(C_W2BT, 0, P, slice(H + MSL[m].start, H + MSL[m].stop)),
                                 t_v1[:, FSL[1]], start=False, stop=True)
                nc.tensor.matmul(p_b[:, FSL[m]], wsl(C_W2AT, 0, P, MSL[m]),
                                 t_v1[:, FSL[0]], start=True, stop=False)
                nc.tensor.matmul(p_b[:, FSL[m]],
                                 wsl(C_W2AT, 0, P, slice(H + MSL[m].start, H + MSL[m].stop)),
                                 t_v1[:, FSL[1]], start=False, stop=True)

            t_z12e = act.tile([P, 2 * N], F32, name="t_z12e")
            t_z22e = act.tile([P, 2 * N], F32, name="t_z22e")
            for m in range(2):
                nc.scalar.activation(t_z12e[:, FSL[m]], p_z12[:, FSL[m]], AF.Identity,
                                     bias=bcol(1, m), scale=1.0)
                nc.vector.tensor_scalar(t_z22e[:, FSL[m]], p_z22[:, FSL[m]],
                                        bcol(2, m), None, op0=ADD)
            for m in range(2):
                nc.scalar.activation(t_z23e[:, FSL[m]], p_z23[:, FSL[m]], AF.Identity,
                                     bias=bcol(4, m), scale=1.0)
            t_y2 = act.tile([P, 2 * N], F32R, name="t_y2")
            t_t1 = act.tile([P, 2 * N], F32R, name="t_t1")
            t_t2 = act.tile([P, 2 * N], F32R, name="t_t2")
            for m in range(2):
                nc.vector.tensor_mul(t_t1[:, FSL[m]], t_z12e[:, FSL[m]], p_a[:, FSL[m]])
            for m in range(2):
                nc.vector.tensor_mul(t_t2[:, FSL[m]], t_z22e[:, FSL[m]], p_b[:, FSL[m]])
            for m in range(2):
                nc.gpsimd.tensor_mul(t_y2[:, FSL[m]], t_z12e[:, FSL[m]], t_z22e[:, FSL[m]])

            # ---- phase 3: layer 3 (u3s joins here to ease PSUM pressure) ----
            p_z13 = pp.tile([P, 2 * N], F32, name="p_z13", tag="ps")
            p_g = pp.tile([P, 2 * N], F32, name="p_g", tag="ps")
            p_u3s = pp.tile([P, 2 * N], F32, name="p_u3s", tag="ps")
            for m in range(2):
                nc.tensor.matmul(p_u3s[:, FSL[m]], wsl(C_W3X, NOBS, P, MSL[m]),
                                 t_xx[NOBS:P, :], start=True, stop=True)
            for m in range(2):
                nc.tensor.matmul(p_z13[:, FSL[m]], wsl(C_W3AT, 0, P, MSL[m]),
                                 t_y2[:, FSL[0]], start=True, stop=False)
                nc.tensor.matmul(p_z13[:, FSL[m]],
                                 wsl(C_W3AT, 0, P, slice(H + MSL[m].start, H + MSL[m].stop)),
                                 t_y2[:, FSL[1]], start=False, stop=True)
                nc.tensor.matmul(p_g[:, FSL[m]], wsl(C_W3AT, 0, P, MSL[m]),
                                 t_t1[:, FSL[0]], start=True, stop=False)
                nc.tensor.matmul(p_g[:, FSL[m]],
                                 wsl(C_W3AT, 0, P, slice(H + MSL[m].start, H + MSL[m].stop)),
                                 t_t1[:, FSL[1]], start=False, stop=False)
                nc.tensor.matmul(p_g[:, FSL[m]], wsl(C_W3AT, 0, P, MSL[m]),
                                 t_t2[:, FSL[0]], start=False, stop=False)
                nc.tensor.matmul(p_g[:, FSL[m]],
                                 wsl(C_W3AT, 0, P, slice(H + MSL[m].start, H + MSL[m].stop)),
                                 t_t2[:, FSL[1]], start=False, stop=True)

            t_z13e = act.tile([P, 2 * N], F32, name="t_z13e")
            for m in range(2):
                nc.scalar.activation(t_z13e[:, FSL[m]], p_z13[:, FSL[m]], AF.Identity,
                                     bias=bcol(3, m), scale=1.0)
            t_y3 = act.tile([P, 2 * N], F32R, name="t_y3")
            t_t3 = act.tile([P, 2 * N], F32R, name="t_t3")
            t_t4 = act.tile([P, 2 * N], F32R, name="t_t4")
            for m in range(2):
                nc.vector.tensor_mul(t_t3[:, FSL[m]], t_z13e[:, FSL[m]], p_u3s[:, FSL[m]])
            for m in range(2):
                nc.gpsimd.tensor_mul(t_y3[:, FSL[m]], t_z13e[:, FSL[m]], t_z23e[:, FSL[m]])
            for m in range(2):
                nc.vector.tensor_mul(t_t4[:, FSL[m]], t_z23e[:, FSL[m]], p_g[:, FSL[m]])
            nc.sync.dma_start(out=d_y[:], in_=t_y3[:])

            # ---- phase 4: wout contractions (M=1) ----
            p_bout = pp.tile([1, N], F32, name="p_bout", tag="ps")
            p_bd = pp.tile([1, N], F32, name="p_bd", tag="ps")
            wo = [t_bigw[:, C_WOUT + k:C_WOUT + k + 1] for k in range(2)]
            nc.tensor.matmul(p_bd[0:1, :], wo[0], t_t3[:, FSL[0]], start=True, stop=False)
            nc.tensor.matmul(p_bd[0:1, :], wo[1], t_t3[:, FSL[1]], start=False, stop=False)
            nc.tensor.matmul(p_bout[0:1, :], wo[0], t_y3[:, FSL[0]], start=True, stop=False)
            nc.tensor.matmul(p_bout[0:1, :], wo[1], t_y3[:, FSL[1]], start=False, stop=True)
            nc.tensor.matmul(p_bd[0:1, :], wo[0], t_t4[:, FSL[0]], start=False, stop=False)
            nc.tensor.matmul(p_bd[0:1, :], wo[1], t_t4[:, FSL[1]], start=False, stop=True)

            t_brow = act.tile([1, N], F32, name="t_brow")
            t_bdrow = act.tile([1, N], F32, name="t_bdrow")
            nc.scalar.copy(t_brow[:], p_bout[0:1, :])
            nc.vector.tensor_copy(t_bdrow[:], p_bd[0:1, :])
            nc.sync.dma_start(out=d_b[:], in_=t_brow[:])
            nc.sync.dma_start(out=d_bd[:], in_=t_bdrow[:])

    nc.compile()
    return nc


def kernel(x, xdot, w1, b1, w2a, b2a, w2b, b2b, w3a, b3a, w3s, b3s, wout, scalar):
    from concourse.bass_utils import run_bass_kernel_spmd

    global LAST
    if "nc" not in _CACHE:
        _CACHE["nc"] = _build()
    nc = _CACHE["nc"]

    f = np.float32
    x = np.asarray(x, f)
    xdot = np.asarray(xdot, f)
    sval = np.asarray(scalar, f).reshape(-1)[0]

    xt_full = np.ascontiguousarray(x.T)
    xdt_full = np.ascontiguousarray(xdot.T)

    def ksplit(w):  # [H, H] -> [128, 512] (k0 | k1)
        wt = np.asarray(w, f).T
        return np.concatenate([wt[:P], wt[P:]], axis=1)

    bigw = np.zeros((P, C_TOT), f)
    bigw[:, C_W2AT:C_W2AT + 2 * H] = ksplit(w2a)
    bigw[:, C_W2BT:C_W2BT + 2 * H] = ksplit(w2b)
    bigw[:, C_W3AT:C_W3AT + 2 * H] = ksplit(w3a)
    w1t = np.asarray(w1, f).T
    w3st = np.asarray(w3s, f).T
    bigw[:NOBS, C_W1X:C_W1X + H] = w1t
    bigw[NOBS:, C_W1X:C_W1X + H] = w1t
    bigw[:NOBS, C_W3X:C_W3X + H] = w3st
    bigw[NOBS:, C_W3X:C_W3X + H] = w3st
    wo = np.asarray(wout, f)[0]
    bigw[:, C_WOUT] = wo[:P]
    bigw[:, C_WOUT + 1] = wo[P:]
    for j, bias in enumerate((b1, b2a, b2b, b3a, b3s)):
        bb = np.asarray(bias, f)
        bigw[:, C_BC + 2 * j] = bb[:P]
        bigw[:, C_BC + 2 * j + 1] = bb[P:]
    bigw = np.ascontiguousarray(bigw)

    in_maps = []
    for c in range(N_CORES):
        sl = slice(c * BL, (c + 1) * BL)
        xx = np.empty((P, BL), f)
        xx[:NOBS] = xt_full[:, sl]
        xx[NOBS:] = xdt_full[:, sl]
        in_maps.append({"xx": xx, "bigw": bigw})

    res = run_bass_kernel_spmd(
        nc, in_maps, core_ids=list(range(N_CORES)),
        trace=TRACE, **TRACE_KW)
    LAST = res

    yb = np.empty((B, 1), f)
    ybdot = np.empty((B,), f)
    yfull = np.empty((B, H), f)
    for c in range(N_CORES):
        sl = slice(c * BL, (c + 1) * BL)
        r = res.results[c]
        yb[sl, 0] = r["bdram"][0]
        ybdot[sl] = r["bddram"][0]
        yfull[sl, :P] = r["ydram"][:, :BL].T
        yfull[sl, P:] = r["ydram"][:, BL:].T
    yyfull = np.broadcast_to(np.float32(sval), (B, NOBS)).copy()
    yyfull += x * 0
    return yb, ybdot, yfull, yyfull
